# revision 1
# baseline (speedup 1.0000x reference)
"""Trainium2 Bass kernel for nn_MemoryGame (scatter_memory).

Math (see reference):
    P = 8192, T = 4 timesteps, N_ITER = 50 attractor iterations.
    per t: h0 = f_p(tile(g_t, 128));  50x: h = f_p(kappa*h + h*(h@M))
           p = outer(x_t, g_t).ravel()
           loss_t = sum|p - h|
           M = lamda*M + yita*outer(p+h, p-h)
    output = sum_t loss_t   (scalar, fp32)

Distribution: M column-sharded over 8 cores (core k owns columns
[k*1024,(k+1)*1024)).  Each core computes its 1024-slice of a = h@M and
an AllGather rebuilds the full h each iteration.

Numerics (measured on CPU, exact-arithmetic sim): storing M in fp16,
rounding h to fp16 each iteration, and accumulating the DVE-side
partials in fp16 all land the final loss within ~5e-4 of the fp32
reference (tolerance 2e-2), so the whole 16 MiB fp16 shard stays
SBUF-resident: ZERO per-iteration HBM traffic.

Per iteration the GEMV is split two ways:
  - contraction chunks (64 of [128 rows, 1024 cols]): N_PE of them run on
    the PE as single-pass fp16 matmuls (h16 stationary, M16 moving);
    the rest accumulate on the DVE via fused scalar_tensor_tensor
    (acc16 = mc*h_c + acc16), partition-reduced by a ones-vector matmul
    into the same PSUM bank as the PE's partials.
  - output columns are processed half0 then half1, so half0's pointwise
    + AllGather launch ~mid-iteration and overlap half1's compute.

Layout: contraction index i = g(c)*128 + p (chunk-major, permuted so
chunks 0-31 hold rows with (i mod 1024) < 512 = the columns carried by
the half-0 AllGather).  The gathered [1,4096] buffer then IS the h
values in [32 chunks, 128 p] order: one contiguous DMA + a PE transpose
+ an ACT fp16 copy rebuild h16_sb[:, chunk] with no strided traffic.
Group-0 chunks only need AG0, so the next iteration's group-0 work
starts while AG1 is still in flight (deferred-assembly software
pipeline); the Hebbian update is one fused DVE op per chunk.
"""

import os
import numpy as np

N_CORES = 8
P_DIM = 8192
NXD, NGD = 128, 64
T_STEPS = 4
N_ITER = 50
KAPPA, LAMDA, YITA = 0.8, 0.9, 0.1
NEG = 0.01

NCHUNK = 64                 # contraction chunks (128 rows each)
J_LOC = P_DIM // N_CORES    # 1024 columns per core
ND_G0 = 10                  # DVE chunks in group0 (starts early, off asm0)
ND_G1 = 4                   # DVE chunks in group1 (kept small: group1 work sits
                            # on the post-asm1 critical path)

_cache = {}


def g_perm(c):
    """chunk -> row-block permutation: chunks 0-31 land in column-half0."""
    if c < 32:
        return (c // 4) * 8 + (c % 4)
    c -= 32
    return (c // 4) * 8 + 4 + (c % 4)


# chunk engine assignment: within each group the first chunks go to the
# PE, the last ND2 to the DVE.
PE_G0 = list(range(0, 32 - ND_G0))
DVE_G0 = list(range(32 - ND_G0, 32))
PE_G1 = list(range(32, 64 - ND_G1))
DVE_G1 = list(range(64 - ND_G1, 64))


def _f_p(v):
    c = np.clip(v, -1.0, 1.0)
    return np.where(c >= 0, c, NEG * c).astype(np.float32)


def build_program(debug_h=False, n_iter=None, t_run=None):
    import concourse.bacc as bacc
    import concourse.mybir as mybir
    import concourse.tile as tile

    if n_iter is None:
        n_iter = N_ITER
    if t_run is None:
        t_run = T_STEPS

    f32 = mybir.dt.float32
    f16 = mybir.dt.float16
    ALU = mybir.AluOpType

    nc = bacc.Bacc(None, target_bir_lowering=False, num_devices=N_CORES)

    AF = mybir.ActivationFunctionType
    # register KAPPA so activation(bias=KAPPA) finds a const AP
    kapc = nc.alloc_sbuf_tensor("const-kappa", [128, 1], f32)
    nc.gpsimd.memset(kapc.ap(), float(KAPPA))
    nc.const_aps.aps[(f32, float(KAPPA))] = kapc.ap()
    nc.all_engine_barrier()

    # ---- I/O ----
    m16_in = nc.dram_tensor("m16_in", [128, NCHUNK * J_LOC], f16, kind="ExternalInput")
    h0_sb_in = nc.dram_tensor("h0_sb_in", [T_STEPS, 128, NGD], f32, kind="ExternalInput")
    h0_row_in = nc.dram_tensor("h0_row_in", [T_STEPS, 1, J_LOC], f32, kind="ExternalInput")
    p_sb_in = nc.dram_tensor("p_sb_in", [T_STEPS, 128, NGD], f32, kind="ExternalInput")
    p_loc_in = nc.dram_tensor("p_loc_in", [T_STEPS, 1, J_LOC], f32, kind="ExternalInput")
    ones_col_in = nc.dram_tensor("ones_col_in", [128, 1], f16, kind="ExternalInput")
    ones_row_in = nc.dram_tensor("ones_row_in", [1, 128], f16, kind="ExternalInput")
    ident_in = nc.dram_tensor("ident_in", [32, 32], f32, kind="ExternalInput")
    loss_out = nc.dram_tensor("loss_out", [1, 1], f32, kind="ExternalOutput")
    if debug_h:
        hdbg_out = nc.dram_tensor("hdbg_out", [t_run * n_iter, 1, J_LOC], f32,
                                  kind="ExternalOutput")

    with tile.TileContext(nc) as tc:
        with (
            tc.tile_pool(name="state_pool", bufs=1) as state_pool,
            tc.tile_pool(name="h_pool", bufs=3) as h_pool,
            tc.tile_pool(name="hr_pool", bufs=2) as hr_pool,
            tc.tile_pool(name="acc_pool", bufs=2) as acc_pool,
            tc.tile_pool(name="pw_pool", bufs=3) as pw_pool,
            tc.tile_pool(name="hT_pool", bufs=2) as hT_pool,
            tc.tile_pool(name="psum_pool", bufs=2, space="PSUM") as psum_pool,
            tc.tile_pool(name="tr_psum_pool", bufs=2, space="PSUM") as tr_psum_pool,
            tc.tile_pool(name="vb_psum_pool", bufs=2, space="PSUM") as vb_psum_pool,
            tc.tile_pool(name="dram_pool", bufs=1, space="DRAM") as dram_pool,
        ):
            # ---- persistent SBUF state ----
            m16 = state_pool.tile([128, NCHUNK * J_LOC], f16)
            v_bcast16 = state_pool.tile([128, J_LOC], f16)
            u_eta = state_pool.tile([128, NGD], f32)
            p_sb = state_pool.tile([128, NGD], f32)
            ones_col16 = state_pool.tile([128, 1], f16)
            ones_row16 = state_pool.tile([1, 128], f16)
            ident32 = state_pool.tile([32, 32], f32)
            loss_acc = state_pool.tile([1, 1], f32)
            loss_tmp = state_pool.tile([1, 1], f32)

            cc_in = [dram_pool.tile([1, 512], f32, name=f"cc_in{h}", tag=f"cc_in{h}")
                     for h in range(2)]

            # ---- init ----
            nc.gpsimd.memset(loss_acc[:], 0.0)
            nc.sync.dma_start(ones_col16[:], ones_col_in[:])
            nc.sync.dma_start(ones_row16[:], ones_row_in[:])
            nc.sync.dma_start(ident32[:], ident_in[:])
            n_ld = 8
            step = (NCHUNK * J_LOC) // n_ld
            for i in range(n_ld):
                nc.sync.dma_start(m16[:, i * step:(i + 1) * step],
                                  m16_in[:, i * step:(i + 1) * step])

            def make_asm(cc_out, h32_next, h16_next, half, t, it):
                """Deferred assembly: DMA gathered h -> transpose -> copies."""
                def emit():
                    hT = hT_pool.tile([32, 128], f32, tag="hT",
                                      name=f"hT_{t}_{it}_{half}")
                    cc_v = cc_out[:].rearrange("o (q p) -> (o q) p", p=128)
                    nc.sync.dma_start(hT[:], cc_v)
                    tr = tr_psum_pool.tile([128, 32], f32, tag="tr",
                                           name=f"tr_{t}_{it}_{half}")
                    nc.tensor.transpose(tr[:], hT[:], ident32[:])
                    cols = slice(half * 32, (half + 1) * 32)
                    nc.scalar.activation(h32_next[:, cols], tr[:], AF.Lrelu,
                                         alpha=float(NEG))
                    nc.scalar.activation(h16_next[:, cols], tr[:], AF.Lrelu,
                                         alpha=float(NEG))
                return emit

            for t in range(t_run):
                scale_t = float(LAMDA ** t)

                h32 = h_pool.tile([128, NGD], f32, tag="h32", name=f"h32_{t}_0")
                nc.sync.dma_start(h32[:], h0_sb_in[t])
                h16 = h_pool.tile([128, NGD], f16, tag="h16", name=f"h16_{t}_0")
                nc.vector.tensor_copy(h16[:], h32[:])
                h_row = hr_pool.tile([1, J_LOC], f32, tag="hr", name=f"hr_{t}_0")
                nc.sync.dma_start(h_row[:], h0_row_in[t])

                pending = [None, None]

                for it in range(n_iter):
                    with nc.named_scope(f"iter_t{t}_i{it}"):
                        acc_ps = psum_pool.tile([1, J_LOC], f32, tag="acc",
                                                name=f"acc_{t}_{it}")
                        acc16 = acc_pool.tile([128, J_LOC], f16, tag="acc16",
                                              name=f"acc16_{t}_{it}")
                        h32_next = h_pool.tile([128, NGD], f32, tag="h32",
                                               name=f"h32_{t}_{it + 1}")
                        h16_next = h_pool.tile([128, NGD], f16, tag="h16",
                                               name=f"h16_{t}_{it + 1}")
                        h_new = hr_pool.tile([1, J_LOC], f32, tag="hr",
                                             name=f"hr_{t}_{it + 1}")

                        def dve_block(chunks, half, init):
                            cs = slice(half * 512, (half + 1) * 512)
                            for n, c in enumerate(chunks):
                                mc = m16[:, c * J_LOC + half * 512:
                                         c * J_LOC + half * 512 + 512]
                                hcol = h32[:, c:c + 1]
                                if init and n == 0:
                                    nc.vector.tensor_scalar_mul(acc16[:, cs], mc, hcol)
                                else:
                                    nc.vector.scalar_tensor_tensor(
                                        acc16[:, cs], mc, hcol, acc16[:, cs],
                                        ALU.mult, ALU.add)

                        def pe_block(chunks, half, start):
                            cs = slice(half * 512, (half + 1) * 512)
                            for n, c in enumerate(chunks):
                                mc = m16[:, c * J_LOC + half * 512:
                                         c * J_LOC + half * 512 + 512]
                                nc.tensor.matmul(acc_ps[:, cs], h16[:, c:c + 1], mc,
                                                 start=(start and n == 0), stop=False,
                                                 skip_group_check=True)

                        def finish_half(half):
                            cs = slice(half * 512, (half + 1) * 512)
                            # partition-reduce the DVE accumulator into the bank
                            nc.tensor.matmul(acc_ps[:, cs], ones_col16[:],
                                             acc16[:, cs], start=False, stop=True,
                                             skip_group_check=True)
                            # pointwise: h = f_p(h*(lamda^t*raw + kappa))
                            s_t = pw_pool.tile([1, 512], f32, tag="pw",
                                               name=f"s_{t}_{it}_{half}")
                            nc.scalar.activation(s_t[:], acc_ps[:, cs], AF.Identity,
                                                 bias=float(KAPPA), scale=scale_t)
                            w = pw_pool.tile([1, 512], f32, tag="pw",
                                             name=f"w_{t}_{it}_{half}")
                            nc.vector.tensor_tensor(w[:], h_row[:, cs], s_t[:],
                                                    ALU.mult)
                            wc = pw_pool.tile([1, 512], f32, tag="pw",
                                              name=f"wc_{t}_{it}_{half}")
                            nc.vector.tensor_scalar(wc[:], w[:], 1.0, -1.0,
                                                    ALU.min, ALU.max)
                            # exchange the clipped pre-lrelu values; lrelu is
                            # applied by the asm copies and locally below
                            nc.sync.dma_start(cc_in[half][:], wc[:])
                            nc.scalar.activation(h_new[:, cs], wc[:], AF.Lrelu,
                                                 alpha=float(NEG))
                            cc_out = dram_pool.tile([1, 4096], f32,
                                                    addr_space="Shared",
                                                    name=f"cc_out_{t}_{it}_{half}",
                                                    tag=f"cc_out_{t}_{it}_{half}")
                            nc.gpsimd.collective_compute(
                                "AllGather", ALU.bypass,
                                replica_groups=[list(range(N_CORES))],
                                ins=[cc_in[half][:].opt()],
                                outs=[cc_out[:].opt()],
                            )
                            return cc_out

                        # --- software-pipelined emission ---
                        if pending[0] is not None:
                            pending[0]()
                        dve_block(DVE_G0, 0, init=True)
                        pe_block(PE_G0, 0, start=True)
                        if pending[1] is not None:
                            pending[1]()
                        dve_block(DVE_G1, 0, init=False)
                        pe_block(PE_G1, 0, start=False)
                        cc0 = finish_half(0)
                        dve_block(DVE_G0, 1, init=True)
                        pe_block(PE_G0, 1, start=True)
                        dve_block(DVE_G1, 1, init=False)
                        pe_block(PE_G1, 1, start=False)
                        cc1 = finish_half(1)
                        if debug_h:
                            nc.sync.dma_start(hdbg_out[t * n_iter + it], h_new[:])

                        pending = [make_asm(cc0, h32_next, h16_next, 0, t, it),
                                   make_asm(cc1, h32_next, h16_next, 1, t, it)]
                        h32 = h32_next
                        h16 = h16_next
                        h_row = h_new

                # ---- timestep tail: final assembly, loss, Hebbian update ----
                pending[0]()
                pending[1]()
                p_loc = pw_pool.tile([1, J_LOC], f32, tag="pw", name=f"ploc_{t}")
                nc.sync.dma_start(p_loc[:], p_loc_in[t])
                v_row = pw_pool.tile([1, J_LOC], f32, tag="pw", name=f"vrow_{t}")
                nc.vector.tensor_tensor(v_row[:], p_loc[:], h_row[:], ALU.subtract)
                nc.vector.tensor_reduce(loss_tmp[:], v_row[:],
                                        mybir.AxisListType.X, ALU.add,
                                        apply_absolute_value=True)
                nc.vector.tensor_tensor(loss_acc[:], loss_acc[:], loss_tmp[:], ALU.add)

                if t < t_run - 1:
                    coef = float(YITA / (LAMDA ** (t + 1)))
                    nc.sync.dma_start(p_sb[:], p_sb_in[t])
                    nc.vector.tensor_tensor(u_eta[:], p_sb[:], h32[:], ALU.add)
                    nc.vector.tensor_scalar_mul(u_eta[:], u_eta[:], coef)
                    v16_row = pw_pool.tile([1, J_LOC], f16, tag="pw16",
                                           name=f"v16_{t}")
                    nc.vector.tensor_copy(v16_row[:], v_row[:])
                    for half in range(2):
                        cs = slice(half * 512, (half + 1) * 512)
                        vb_ps = vb_psum_pool.tile([128, 512], f32, tag="vb",
                                                  name=f"vb_{t}_{half}")
                        nc.tensor.matmul(vb_ps[:], ones_row16[:], v16_row[:, cs],
                                         start=True, stop=True)
                        nc.vector.tensor_copy(v_bcast16[:, cs], vb_ps[:])
                    for c in range(NCHUNK):
                        mc = m16[:, c * J_LOC:(c + 1) * J_LOC]
                        nc.vector.scalar_tensor_tensor(mc, v_bcast16[:],
                                                       u_eta[:, c:c + 1], mc,
                                                       ALU.mult, ALU.add)

            nc.sync.dma_start(loss_out[:], loss_acc[:])

    nc.compile()
    return nc


def prepare_inputs(x, g, M0):
    """Host-side sharding/layout prep. Returns list of per-core input maps."""
    x = np.asarray(x, dtype=np.float32)
    g = np.asarray(g, dtype=np.float32)
    M0 = np.ascontiguousarray(np.asarray(M0, dtype=np.float32))

    perm = np.array([g_perm(c) for c in range(NCHUNK)])
    Mv = M0.reshape(NCHUNK, 128, P_DIM)[perm]          # [c, p, col]

    h0_sb = np.zeros((T_STEPS, 128, NGD), np.float32)
    p_sb = np.zeros((T_STEPS, 128, NGD), np.float32)
    h0_flat = np.zeros((T_STEPS, P_DIM), np.float32)
    p_flat = np.zeros((T_STEPS, P_DIM), np.float32)
    for t in range(T_STEPS):
        h0 = _f_p(np.tile(g[t], NXD))
        p = np.outer(x[t], g[t]).reshape(P_DIM).astype(np.float32)
        h0_flat[t] = h0
        p_flat[t] = p
        h0_sb[t] = h0.reshape(NCHUNK, 128)[perm].T
        p_sb[t] = p.reshape(NCHUNK, 128)[perm].T

    ones_col = np.ones((128, 1), np.float16)
    ones_row = np.ones((1, 128), np.float16)
    ident = np.eye(32, dtype=np.float32)

    in_maps = []
    for k in range(N_CORES):
        shard = Mv[:, :, k * J_LOC:(k + 1) * J_LOC]    # [64, 128, 1024]
        m16 = np.ascontiguousarray(shard.transpose(1, 0, 2)).reshape(
            128, NCHUNK * J_LOC).astype(np.float16)
        in_maps.append({
            "m16_in": m16,
            "h0_sb_in": h0_sb,
            "h0_row_in": h0_flat[:, k * J_LOC:(k + 1) * J_LOC].reshape(
                T_STEPS, 1, J_LOC).copy(),
            "p_sb_in": p_sb,
            "p_loc_in": p_flat[:, k * J_LOC:(k + 1) * J_LOC].reshape(
                T_STEPS, 1, J_LOC).copy(),
            "ones_col_in": ones_col,
            "ones_row_in": ones_row,
            "ident_in": ident,
        })
    return in_maps


def kernel(x, g, M0):
    from concourse.bass_utils import run_bass_kernel_spmd

    in_maps = prepare_inputs(x, g, M0)
    if "nc" not in _cache:
        _cache["nc"] = build_program()
    nc = _cache["nc"]
    trace = bool(int(os.environ.get("MG_TRACE", "0")))
    res = run_bass_kernel_spmd(nc, in_maps, core_ids=list(range(N_CORES)),
                               trace=trace)
    _cache["last_result"] = res
    total = np.float32(0.0)
    for k in range(N_CORES):
        total += res.results[k]["loss_out"][0, 0]
    return np.float32(total)



# revision 2
# speedup vs baseline: 1.0185x; 1.0185x over previous
"""Trainium2 Bass kernel v2 for nn_MemoryGame (scatter_memory).

Math (see reference):
    P = 8192, T = 4 timesteps, N_ITER = 50 attractor iterations.
    per t: h0 = f_p(tile(g_t, 128));  50x: h = f_p(kappa*h + h*(h@M))
           p = outer(x_t, g_t).ravel();  loss_t = sum|p - h|
           M = lamda*M + yita*outer(p+h, p-h)
    output = sum_t loss_t.

Design:
  * M0 stored fp8e4m3 (scaled S_M), NEVER updated on device. The Hebbian
    rank-1 updates are applied analytically:
        a = h@M_t = lam^t (h@M0) + sum_k lam^(t-1-k) eta (h.u'_k) v'_k
    u',v' stored exactly in fp16 -> no update quantization noise and no
    t-boundary M rewrites.
  * GEMV on the PE in DoubleRow fp8 mode: pair pc folds chunks
    (pc, pc+16 mod-32-group), K=256 per matmul, 64 matmuls of [1,512].
  * Column-sharded over 8 cores; two 512-col halves AllGather'd separately
    (fp16 payload, scaled 1/AG_DOWN) so the next iteration's first chunk
    group starts while the second AG is in flight.
  * Post-AG: contiguous DMA + PE transpose -> [128,32] tile; pointwise and
    the rank-k correction (v as TILES, d broadcast via a ones-matmul) all
    happen in tile form; corrections are precomputed off the critical path
    during the GEMV matmuls.

Chunk mapping: global h index i <-> (chunk c, partition p):
    c < 32 : k=c//4, i = 1024k + (c%4)*128 + p      (AG0: local cols 0..511)
    c >= 32: k=(c-32)//4, i = 1024k + 512 + ((c-32)%4)*128 + p  (AG1)
"""

import os
import numpy as np
import ml_dtypes

N_CORES = 8
P_DIM = 8192
NXD, NGD = 128, 64
T_STEPS = 4
N_ITER = 50
KAPPA, LAMDA, YITA = 0.8, 0.9, 0.1
NEG = 0.01

S_M = 32.0       # fp8 scale for M0
S_H = 16.0       # fp8/fp16 scale for h
AG_DOWN = 256.0  # psum divided by this before the fp16 AllGather
J_LOC = P_DIM // N_CORES
E4NP = ml_dtypes.float8_e4m3

USE_RESIDUAL = bool(int(os.environ.get("MG_RESIDUAL", "0")))
N_JUNK = int(os.environ.get("MG_JUNK", "0"))   # PE p-state warmer matmuls

_cache = {}


def chunk_index_map():
    idx = np.zeros((64, 128), np.int64)
    for c in range(64):
        pr = np.arange(128)
        if c < 32:
            idx[c] = 1024 * (c // 4) + (c % 4) * 128 + pr
        else:
            idx[c] = 1024 * ((c - 32) // 4) + 512 + ((c - 32) % 4) * 128 + pr
    return idx


IDX = chunk_index_map()


def to_tile(vec):
    """[P] -> [128, 64] tile (partition p, chunk c)."""
    return vec[IDX].T.copy()


def to_pair_tile(vec):
    """[P] -> [128, 2, 32]: pair pc = chunks (32*(pc//16)+pc%16+16*i)."""
    t = vec[IDX]
    out = np.zeros((128, 2, 32), t.dtype)
    for pc in range(32):
        for i in range(2):
            c = 32 * (pc // 16) + (pc % 16) + 16 * i
            out[:, i, pc] = t[c]
    return out


def _f_p(v):
    c = np.clip(v, -1.0, 1.0)
    return np.where(c >= 0, c, NEG * c).astype(np.float32)


def build_vs(k):
    """v-tile scale for correction k: true-units eta/lam^{k+1}."""
    return float(YITA / LAMDA ** (k + 1))


def build_program(debug_h=False, n_iter=None, t_run=None):
    import concourse.bacc as bacc
    import concourse.mybir as mybir
    import concourse.tile as tile

    if n_iter is None:
        n_iter = N_ITER
    if t_run is None:
        t_run = T_STEPS

    f32 = mybir.dt.float32
    f16 = mybir.dt.float16
    f8 = mybir.dt.float8e4
    ALU = mybir.AluOpType
    AF = mybir.ActivationFunctionType
    DR = mybir.MatmulPerfMode.DoubleRow

    nc = bacc.Bacc(None, target_bir_lowering=False, num_devices=N_CORES)
    nc.all_engine_barrier()

    # ---- I/O ----
    m8_in = nc.dram_tensor("m8_in", [128, 64, J_LOC], f8, kind="ExternalInput")
    if USE_RESIDUAL:
        r8_in = nc.dram_tensor("r8_in", [128, 64, J_LOC], f8, kind="ExternalInput")
        h0_8r_in = nc.dram_tensor("h0_8r_in", [T_STEPS, 128, 2, 32], f8,
                                  kind="ExternalInput")
    h0_8_in = nc.dram_tensor("h0_8_in", [T_STEPS, 128, 2, 32], f8, kind="ExternalInput")
    h0_16_in = nc.dram_tensor("h0_16_in", [T_STEPS, 128, 64], f16, kind="ExternalInput")
    p_sb_in = nc.dram_tensor("p_sb_in", [T_STEPS, 128, 64], f32, kind="ExternalInput")
    ident_in = nc.dram_tensor("ident_in", [32, 32], f16, kind="ExternalInput")
    ones32_in = nc.dram_tensor("ones32_in", [128, 1], f32, kind="ExternalInput")
    loss_out = nc.dram_tensor("loss_out", [1, 1], f32, kind="ExternalOutput")
    if debug_h:
        hdbg_out = nc.dram_tensor("hdbg_out", [t_run * n_iter, 128, 64], f16,
                                  kind="ExternalOutput")

    # ag16 carries TRUE a-units: ag = lam^t/(S_M*S_H) * psum.
    AGC_t = {t: float(LAMDA ** t / (S_M * S_H)) for t in range(T_STEPS)}
    LAM_t = {t: float(LAMDA ** t) for t in range(T_STEPS)}

    with tile.TileContext(nc) as tc:
        with (
            tc.tile_pool(name="state", bufs=1) as state,
            tc.tile_pool(name="h8_pool", bufs=3) as h8_pool,
            tc.tile_pool(name="h16_pool", bufs=3) as h16_pool,
            tc.tile_pool(name="a_pool", bufs=4) as a_pool,
            tc.tile_pool(name="pw_pool", bufs=4) as pw_pool,
            tc.tile_pool(name="d_pool", bufs=2) as d_pool,
            tc.tile_pool(name="b_pool", bufs=2) as b_pool,
            tc.tile_pool(name="acc_psum", bufs=2, space="PSUM") as acc_psum,
            tc.tile_pool(name="d_psum", bufs=2, space="PSUM") as d_psum,
            tc.tile_pool(name="tr_psum", bufs=1, space="PSUM") as tr_psum,
            tc.tile_pool(name="b_psum", bufs=1, space="PSUM") as b_psum,
            tc.tile_pool(name="j_psum", bufs=1, space="PSUM") as j_psum,
            tc.tile_pool(name="dram_pool", bufs=1, space="DRAM") as dram_pool,
        ):
            # ---- persistent SBUF state ----
            m8 = state.tile([128, 64, J_LOC], f8)
            if USE_RESIDUAL:
                r8 = state.tile([128, 64, J_LOC], f8)
            u16 = [state.tile([128, 64], f16, name=f"u16_{k}") for k in range(3)]
            v16t = [state.tile([128, 64], f16, name=f"v16t_{k}") for k in range(3)]
            ones128 = state.tile([128, 128], f16)
            ident = state.tile([32, 32], f16)
            ones32 = state.tile([128, 1], f32)
            p_sb = state.tile([128, 64], f32)
            loss_acc = state.tile([1, 1], f32)

            # ---- init ----
            nc.gpsimd.memset(loss_acc[:], 0.0)
            nc.gpsimd.memset(ones128[:], 1.0)
            nc.sync.dma_start(ident[:], ident_in[:])
            nc.sync.dma_start(ones32[:], ones32_in[:])
            for i in range(8):
                nc.sync.dma_start(m8[:, i * 8:(i + 1) * 8, :],
                                  m8_in[:, i * 8:(i + 1) * 8, :])
                if USE_RESIDUAL:
                    nc.sync.dma_start(r8[:, i * 8:(i + 1) * 8, :],
                                      r8_in[:, i * 8:(i + 1) * 8, :])

            def make_asm(cc_out, h16s_cur, tmp2, h8s_next, h16s_next, h8r_next,
                         half, t, it):
                """Post-AG assembly: DMA + transpose + pointwise (+corr)."""
                cs = slice(half * 32, (half + 1) * 32)
                pcs = slice(half * 16, (half + 1) * 16)

                def emit():
                    with tc.high_priority():
                        _emit_inner()

                def _emit_inner():
                    hT = a_pool.tile([32, 128], f16, tag=f"hT{half}",
                                     name=f"hT_{t}_{it}_{half}")
                    src = cc_out[:].rearrange("o (c p) -> (o c) p", p=128)
                    nc.sync.dma_start(hT[:], src)
                    tr = tr_psum.tile([128, 32], f16, tag=f"tr{half}",
                                      name=f"tr_{t}_{it}_{half}")
                    nc.tensor.transpose(tr[:], hT[:], ident[:])
                    u1 = pw_pool.tile([128, 32], f16, tag=f"u1{half}",
                                      name=f"u1_{t}_{it}_{half}")
                    if tmp2 is None:
                        nc.vector.tensor_scalar_add(u1[:], tr[:], float(KAPPA))
                    else:
                        nc.vector.tensor_tensor(u1[:], tr[:], tmp2[:], ALU.add)
                    w16 = pw_pool.tile([128, 32], f16, tag=f"w{half}",
                                       name=f"w_{t}_{it}_{half}")
                    nc.vector.tensor_tensor(w16[:], u1[:], h16s_cur[:, cs],
                                            ALU.mult)
                    wc16 = pw_pool.tile([128, 32], f16, tag=f"wc{half}",
                                        name=f"wc_{t}_{it}_{half}")
                    nc.vector.tensor_scalar(wc16[:], w16[:], float(S_H),
                                            -float(S_H), ALU.min, ALU.max)
                    # Lrelu via stt: max(0.01*x, x); h8s first (gates the PE)
                    wcv = wc16[:].rearrange("p (i q) -> p i q", i=2)
                    nc.vector.scalar_tensor_tensor(
                        h8s_next[:, :, pcs], wcv, float(NEG), wcv,
                        ALU.mult, ALU.max)
                    if USE_RESIDUAL:
                        nc.vector.tensor_scalar_mul(
                            h8r_next[:, :, pcs], h8s_next[:, :, pcs],
                            1.0 / 16.0)
                    nc.vector.scalar_tensor_tensor(
                        h16s_next[:, cs], wc16[:], float(NEG), wc16[:],
                        ALU.mult, ALU.max)
                return emit

            for t in range(t_run):
                nc.sync.dma_start(p_sb[:], p_sb_in[t])
                h8s = h8_pool.tile([128, 2, 32], f8, tag="h8", name=f"h8_{t}_0")
                nc.sync.dma_start(h8s[:], h0_8_in[t])
                h16s = h16_pool.tile([128, 64], f16, tag="h16", name=f"h16_{t}_0")
                nc.sync.dma_start(h16s[:], h0_16_in[t])
                h8r = None
                if USE_RESIDUAL:
                    h8r = h8_pool.tile([128, 2, 32], f8, tag="h8r",
                                       name=f"h8r_{t}_0")
                    nc.sync.dma_start(h8r[:], h0_8r_in[t])

                pending = [None, None]

                # Schedule floors: the scheduler's collective cost model
                # (15us fixed overhead) otherwise buries the critical
                # assembly/AG-launch ops behind all ready matmuls. PB is the
                # believed period; offsets pin the per-engine ORDER (runtime
                # paces itself via semaphores, so believed times are not
                # real waits).
                PB = 40e-3   # ms

                for it in range(n_iter):
                    gi = t * n_iter + it
                    W = gi * PB
                    with nc.named_scope(f"iter_t{t}_i{it}"):
                        h8s_next = h8_pool.tile([128, 2, 32], f8, tag="h8",
                                                name=f"h8_{t}_{it + 1}")
                        h16s_next = h16_pool.tile([128, 64], f16, tag="h16",
                                                  name=f"h16_{t}_{it + 1}")
                        h8r_next = None
                        if USE_RESIDUAL:
                            h8r_next = h8_pool.tile([128, 2, 32], f8, tag="h8r",
                                                    name=f"h8r_{t}_{it + 1}")

                        m8v = m8[:].rearrange("p (g c) j -> p g c j", g=4)
                        if USE_RESIDUAL:
                            r8v = r8[:].rearrange("p (g c) j -> p g c j", g=4)

                        def mm_list(half):
                            cs = slice(half * 512, (half + 1) * 512)
                            mms = []
                            for pc in range(32):
                                g0, c0 = 2 * (pc // 16), pc % 16
                                mms.append((h8s[:, :, pc:pc + 1],
                                            m8v[:, g0:g0 + 2, c0, cs]))
                                if USE_RESIDUAL:
                                    mms.append((h8r[:, :, pc:pc + 1],
                                                r8v[:, g0:g0 + 2, c0, cs]))
                            return mms

                        def emit_mms(psum, mms, lo, hi, start, stop):
                            n = len(mms)
                            for j in range(lo, hi):
                                lhsT, rhs = mms[j]
                                nc.tensor.matmul(psum[:], lhsT, rhs,
                                                 start=(start and j == lo),
                                                 stop=(stop and j == hi - 1),
                                                 perf_mode=DR,
                                                 skip_group_check=True)

                        def launch_ag(half, psum):
                            with tc.high_priority():
                                ag16 = pw_pool.tile(
                                    [1, 512], f16, tag=f"ag{half}",
                                    name=f"ag_{t}_{it}_{half}")
                                nc.scalar.activation(ag16[:], psum[:], AF.Copy,
                                                     scale=AGC_t[t])
                                cc_in = dram_pool.tile(
                                    [1, 512], f16, name=f"cci_{t}_{it}_{half}",
                                    tag=f"cci_{t}_{it}_{half}")
                                # ACT triggers the DMA itself: same-engine
                                # in-order issue, no cross-engine semaphore.
                                nc.scalar.dma_start(cc_in[:], ag16[:])
                                cc_out = dram_pool.tile(
                                    [1, 4096], f16, addr_space="Shared",
                                    name=f"cco_{t}_{it}_{half}",
                                    tag=f"cco_{t}_{it}_{half}")
                                nc.gpsimd.collective_compute(
                                    "AllGather", ALU.bypass,
                                    replica_groups=[list(range(N_CORES))],
                                    ins=[cc_in[:].opt()],
                                    outs=[cc_out[:].opt()],
                                )
                            return cc_out

                        psum0 = acc_psum.tile([1, 512], f32, tag="acc",
                                              name=f"ps0_{t}_{it}")
                        psum1 = acc_psum.tile([1, 512], f32, tag="acc",
                                              name=f"ps1_{t}_{it}")
                        mms0 = mm_list(0)
                        mms1 = mm_list(1)
                        gmid = len(mms0) // 2

                        # ---------- emission ----------
                        # psum0 group: [G0h0 | G1h0]; psum1 group: [G1h1 |
                        # G0h1]. In-group order is fixed by start/stop, so
                        # the early-ready G0h1 cannot be hoisted ahead of the
                        # asm1-gated work and delay AG0's launch.
                        if pending[0] is not None:
                            pending[0]()
                        emit_mms(psum0, mms0, 0, gmid, start=True, stop=False)
                        if pending[1] is not None:
                            pending[1]()
                        emit_mms(psum0, mms0, gmid, len(mms0), start=False,
                                 stop=True)
                        cc0 = launch_ag(0, psum0)
                        emit_mms(psum1, mms1, gmid, len(mms1), start=True,
                                 stop=False)
                        emit_mms(psum1, mms1, 0, gmid, start=False, stop=True)
                        cc1 = launch_ag(1, psum1)

                        # corrections for the NEXT assembly (uses this
                        # iteration's full h16s) -- off the AG launch path.
                        tmp2 = [None, None]
                        if t > 0:
                            ctx_w = tc.tile_wait_until(W + 17e-3)
                            ctx_w.__enter__()
                        if t > 0:
                            dparts = d_pool.tile([128, 4], f16, tag="dp",
                                                 name=f"dp_{t}_{it}")
                            for k in range(t):
                                prod = d_pool.tile([128, 64], f16, tag=f"pr{k}",
                                                   name=f"pr_{t}_{it}_{k}")
                                nc.vector.tensor_tensor(prod[:], h16s[:],
                                                        u16[k][:], ALU.mult)
                                with nc.allow_low_precision(
                                        reason="d partials bounded ~4e3; "
                                               "fp16 rel err 5e-4 suffices"):
                                    nc.vector.tensor_reduce(
                                        dparts[:, k:k + 1], prod[:],
                                        mybir.AxisListType.X, ALU.add)
                            dbc = d_psum.tile([128, 4], f32, tag="dbc",
                                              name=f"dbc_{t}_{it}")
                            nc.tensor.matmul(dbc[:, 0:t], ones128[:],
                                             dparts[:, 0:t], start=True,
                                             stop=True, skip_group_check=True)
                            d16b = d_pool.tile([128, 4], f32, tag="d16b",
                                               name=f"d16b_{t}_{it}")
                            nc.scalar.activation(d16b[:, 0:t], dbc[:, 0:t],
                                                 AF.Copy)
                            for half in range(2):
                                cs = slice(half * 32, (half + 1) * 32)
                                tm = pw_pool.tile([128, 32], f16,
                                                  tag=f"tm{half}",
                                                  name=f"tm_{t}_{it}_{half}")
                                nc.vector.tensor_scalar_mul(
                                    tm[:], v16t[0][:, cs], d16b[:, 0:1])
                                for k in range(1, t):
                                    nc.vector.scalar_tensor_tensor(
                                        tm[:], v16t[k][:, cs], d16b[:, k:k + 1],
                                        tm[:], ALU.mult, ALU.add)
                                # tmp2 = lam^t * tm + kappa (true s-units)
                                nc.vector.tensor_scalar(tm[:], tm[:], LAM_t[t],
                                                        float(KAPPA), ALU.mult,
                                                        ALU.add)
                                tmp2[half] = tm
                        if t > 0:
                            ctx_w.__exit__(None, None, None)

                        # p-state warmers: keep the PE clock at full speed
                        # through the AG-wait idle window. Junk DR matmuls
                        # into a scratch psum bank; results never read.
                        if N_JUNK:
                            jp = j_psum.tile([1, 512], f32, tag="junk",
                                             name=f"junk_{t}_{it}")
                            for j in range(N_JUNK):
                                nc.tensor.matmul(jp[:], h8s[:, :, 0:1],
                                                 m8v[:, 0:2, 0, 0:512],
                                                 start=True, stop=True,
                                                 perf_mode=DR,
                                                 skip_group_check=True)

                        if debug_h and it > 0:
                            nc.sync.dma_start(hdbg_out[t * n_iter + it - 1],
                                              h16s[:])

                        pending = [
                            make_asm(cc0, h16s, tmp2[0], h8s_next, h16s_next,
                                     h8r_next, 0, t, it),
                            make_asm(cc1, h16s, tmp2[1], h8s_next, h16s_next,
                                     h8r_next, 1, t, it),
                        ]
                        h8s = h8s_next
                        h16s = h16s_next
                        if USE_RESIDUAL:
                            h8r = h8r_next

                # ---- timestep tail ----
                pending[0]()
                pending[1]()
                if debug_h:
                    nc.sync.dma_start(hdbg_out[t * n_iter + n_iter - 1], h16s[:])

                h_tile = b_pool.tile([128, 64], f32, tag="ht", name=f"ht_{t}")
                nc.scalar.activation(h_tile[:], h16s[:], AF.Copy,
                                     scale=1.0 / float(S_H))
                diff = b_pool.tile([128, 64], f32, tag="df", name=f"df_{t}")
                nc.vector.tensor_tensor(diff[:], p_sb[:], h_tile[:], ALU.subtract)
                lparts = b_pool.tile([128, 1], f32, tag="lp", name=f"lp_{t}")
                nc.vector.tensor_reduce(lparts[:], diff[:],
                                        mybir.AxisListType.X, ALU.add,
                                        apply_absolute_value=True)
                psl = b_psum.tile([1, 1], f32, tag="psl", name=f"psl_{t}")
                nc.tensor.matmul(psl[:], ones32[:], lparts[:], start=True,
                                 stop=True, skip_group_check=True)
                nc.vector.tensor_tensor(loss_acc[:], loss_acc[:], psl[:], ALU.add)

                if t < t_run - 1:
                    usum = b_pool.tile([128, 64], f32, tag="us", name=f"us_{t}")
                    nc.vector.tensor_tensor(usum[:], p_sb[:], h_tile[:], ALU.add)
                    nc.vector.tensor_scalar_mul(u16[t][:], usum[:],
                                                1.0 / float(S_H))
                    # v tile = (p - h) * VS_t
                    vt = b_pool.tile([128, 64], f32, tag="vt", name=f"vtb_{t}")
                    nc.vector.tensor_tensor(vt[:], p_sb[:], h_tile[:],
                                            ALU.subtract)
                    nc.vector.tensor_scalar_mul(v16t[t][:], vt[:], build_vs(t))

            nc.sync.dma_start(loss_out[:], loss_acc[:])

    nc.compile()
    return nc


def prepare_inputs(x, g, M0):
    x = np.asarray(x, dtype=np.float32)
    g = np.asarray(g, dtype=np.float32)
    M0 = np.ascontiguousarray(np.asarray(M0, dtype=np.float32))

    Mv = M0[IDX.reshape(-1)]
    Mv = Mv.reshape(64, 128, P_DIM).transpose(1, 0, 2)   # [p, c, col]

    h0_8 = np.zeros((T_STEPS, 128, 2, 32), E4NP)
    h0_16 = np.zeros((T_STEPS, 128, 64), np.float16)
    h0_8r = np.zeros((T_STEPS, 128, 2, 32), E4NP)
    p_sb = np.zeros((T_STEPS, 128, 64), np.float32)
    for t in range(T_STEPS):
        h0 = _f_p(np.tile(g[t], NXD))
        p = np.outer(x[t], g[t]).reshape(P_DIM).astype(np.float32)
        hs16 = (S_H * h0).astype(np.float16)
        h0_16[t] = to_tile(hs16.astype(np.float32)).astype(np.float16)
        h8 = hs16.astype(E4NP)
        h0_8[t] = to_pair_tile(h8.astype(np.float32)).astype(E4NP)
        h8r = (hs16.astype(np.float32) / 16.0).astype(E4NP)
        h0_8r[t] = to_pair_tile(h8r.astype(np.float32)).astype(E4NP)
        p_sb[t] = to_tile(p)

    ident = np.eye(32, dtype=np.float16)
    ones32 = np.ones((128, 1), np.float32)

    in_maps = []
    for k in range(N_CORES):
        shard = Mv[:, :, k * J_LOC:(k + 1) * J_LOC]
        m8 = (S_M * shard).astype(E4NP)
        im = {
            "m8_in": m8,
            "h0_8_in": h0_8,
            "h0_16_in": h0_16,
            "p_sb_in": p_sb,
            "ident_in": ident,
            "ones32_in": ones32,
        }
        if USE_RESIDUAL:
            im["r8_in"] = (16.0 * (S_M * shard - m8.astype(np.float32))).astype(E4NP)
            im["h0_8r_in"] = h0_8r
        in_maps.append(im)
    return in_maps


def kernel(x, g, M0):
    from concourse.bass_utils import run_bass_kernel_spmd

    in_maps = prepare_inputs(x, g, M0)
    if "nc" not in _cache:
        _cache["nc"] = build_program()
    nc = _cache["nc"]
    trace = bool(int(os.environ.get("MG_TRACE", "0")))
    res = run_bass_kernel_spmd(nc, in_maps, core_ids=list(range(N_CORES)),
                               trace=trace)
    _cache["last_result"] = res
    return np.float32(res.results[0]["loss_out"][0, 0])
